# revision 3
# baseline (speedup 1.0000x reference)
"""GATv2Conv forward on 8 Trainium2 NeuronCores (Bass/Tile).

Strategy
--------
Edges are sorted by destination node and packed into "groups" of at most
S=256 edge slots / at most 128 distinct destinations, such that every
destination's edge run lies entirely inside one group.  Groups are split
evenly across the 8 cores, so all softmax segments and output rows are
core-local: no collectives are needed.

Host preprocessing (index plumbing + weight prep only):
  - gathers feat[src]/feat[dst] per edge slot (fp16, K-major transposed),
  - folds a signed per-column scale c_d into the W columns, with columns
    permuted so attn>=0 dims come first (p1 of them):
      c_d = K*attn_d            for attn_d >= 0
      c_d = K*attn_d*0.2        for attn_d <  0   (c_d < 0)
    Then on device, with psum = c_d*(el+er)_d:
      Prelu(psum, alpha=0.2) on the pos block and Prelu(psum, alpha=5.0)
      on the neg block both equal K*attn_d*leakyrelu((el+er)_d, 0.2), so
      score*K = plain sum of the activation output over all 256 dims.
    el is recovered exactly as psum*(1/c) during the PSUM evacuation.

Per core (SPMD, identical program; per-core data differs):
  - per group, a 2-bank PSUM pair [128, 2, 256] (chunk-major, stride 512):
    4 fp16 el matmuls, ONE strided DVE evacuation into the bf16
    aggregation operand V=[1|el], then 4 er matmuls accumulate in place,
  - 2 strided ACT Prelu ops (alpha 0.2 / 5.0) write signed score summands
    into a per-batch bf16 tile; ONE DVE tensor_reduce per batch (4 groups,
    8 chunks) yields score*K per chunk; ONE batched ACT Exp(scale=1/K),
  - per group: bf16 one-hot scatter-add matmul
    psum[rank,:] += Hx^T @ V with Hx[e,k] = (rank_e==k)*ex_e
    (Hx built by one DVE tensor_scalar: is_equal(iota,rank)*ex);
    segment sums land in column 0.  pg tiles are paired (2 groups / 2
    banks) so the reciprocal is one strided DVE op per pair,
  - out rows = psum[:,1:257] * rcp  (ACT copy-mul, fp16 out).

Host unshards by scattering dense group rows to their global node ids and
undoing the attn column permutation (pure indexing; all arithmetic on-device).
"""

import math
import numpy as np

import concourse.bass as bass
import concourse.mybir as mybir
import concourse.tile as tile
from concourse import bacc
from concourse.bass_utils import run_bass_kernel_spmd

F32 = mybir.dt.float32
F16 = mybir.dt.float16
BF16 = mybir.dt.bfloat16
I32 = mybir.dt.int32
AF = mybir.ActivationFunctionType
ALU = mybir.AluOpType
AX = mybir.AxisListType

N_CORES = 8
S = 256            # edge slots per group (2 chunks of 128)
CHUNK = 128
GPB = 4            # groups per batch
CPB = 2 * GPB      # chunks per batch
SC = S * GPB       # edge slots per batch
NEG_SLOPE = 0.2

LAST_RESULTS = None  # BassKernelResults of the most recent run (for test.py)
LAST_NC = None       # compiled Bacc program of the most recent run
LAST_IN_MAPS = None  # per-core input dicts of the most recent run
LAST_BUILD_ARGS = None  # (g_pc, has_bias, p1, inv_k) for rebuilds


# ----------------------------------------------------------------- host prep

def _pack_runs(counts, max_slots=S, max_nodes=128):
    """Greedily pack whole runs (same-dst edge blocks) into groups."""
    n = len(counts)
    grp = np.empty(n, np.int64)
    rank = np.empty(n, np.int64)
    g = used = nodes = 0
    for i in range(n):
        c = counts[i]
        if used + c > max_slots or nodes >= max_nodes:
            g += 1
            used = 0
            nodes = 0
        grp[i] = g
        rank[i] = nodes
        used += c
        nodes += 1
    return grp, rank, g + 1


def _prepare(feat, src, dst):
    """Sort edges by dst, pack into groups, build per-core input arrays."""
    E = dst.shape[0]
    order = np.argsort(dst, kind="stable")
    sd = dst[order].astype(np.int64)
    ss = src[order].astype(np.int64)

    uniq, counts = np.unique(sd, return_counts=True)
    # split pathological runs longer than S so packing can't fail
    need_accum = bool((counts > S).any())
    if need_accum:
        new_uniq, new_counts = [], []
        for u, c in zip(uniq, counts):
            while c > S:
                new_uniq.append(u)
                new_counts.append(S)
                c -= S
            new_uniq.append(u)
            new_counts.append(c)
        uniq = np.array(new_uniq, np.int64)
        counts = np.array(new_counts, np.int64)

    grp, rank, g_tot = _pack_runs(counts)
    n_runs = len(counts)

    starts = np.zeros(n_runs, np.int64)
    np.cumsum(counts[:-1], out=starts[1:])
    grp_first_run = np.searchsorted(grp, np.arange(g_tot))
    grp_start_edge = starts[grp_first_run]

    run_of_edge = np.repeat(np.arange(n_runs), counts)
    e_grp = grp[run_of_edge]
    e_rank = rank[run_of_edge]
    e_slot = e_grp * S + np.arange(E) - grp_start_edge[e_grp]

    g_pc = math.ceil(g_tot / N_CORES)
    g_pc = math.ceil(g_pc / GPB) * GPB          # multiple of GPB per core
    e_slots = g_pc * S
    total_slots = N_CORES * e_slots

    slot_src = np.zeros(total_slots, np.int64)
    slot_dst = np.zeros(total_slots, np.int64)
    slot_rank = np.full(total_slots, -1.0, np.float32)
    slot_src[e_slot] = ss
    slot_dst[e_slot] = sd
    slot_rank[e_slot] = e_rank

    # unshard info per run
    run_core = grp // g_pc
    run_pos = (grp % g_pc) * 128 + rank        # row in the core's dense output

    return dict(
        g_pc=g_pc, e_slots=e_slots, need_accum=need_accum,
        slot_src=slot_src, slot_dst=slot_dst, slot_rank=slot_rank,
        run_core=run_core, run_pos=run_pos, run_node=uniq,
    )


# ------------------------------------------------------------ device program

def _build_program(n_g, has_bias, p1, inv_k, repeat=1):
    """p1: number of attn>=0 dims after the host sign-sort permutation.
    inv_k: 1/K where K is the global scale folded into the W columns."""
    e_slots = n_g * S
    nb = n_g // GPB
    nc = bacc.Bacc("TRN2", target_bir_lowering=False, debug=False,
                   num_devices=N_CORES)
    fsT_d = nc.dram_tensor("fsT", [2, 128, e_slots], F16, kind="ExternalInput").ap()
    fdT_d = nc.dram_tensor("fdT", [2, 128, e_slots], F16, kind="ExternalInput").ap()
    rankT_d = nc.dram_tensor("rankT", [128, n_g * 2], F32, kind="ExternalInput").ap()
    wsrc_d = nc.dram_tensor("wsrc", [2, 128, 256], F16, kind="ExternalInput").ap()
    wdst_d = nc.dram_tensor("wdst", [2, 128, 256], F16, kind="ExternalInput").ap()
    invc_d = nc.dram_tensor("invc", [1, 512], F32, kind="ExternalInput").ap()
    if has_bias:
        bsrc_d = nc.dram_tensor("bsrc", [1, 256], F16, kind="ExternalInput").ap()
        bdst_d = nc.dram_tensor("bdst", [1, 256], F16, kind="ExternalInput").ap()
    dense_d = nc.dram_tensor("dense", [n_g * 128, 256], F16,
                             kind="ExternalOutput").ap()

    with tile.TileContext(nc) as tc:
        with (
            tc.tile_pool(name="const", bufs=1) as cpool,
            tc.tile_pool(name="fsp", bufs=3) as fs_pool,
            tc.tile_pool(name="fdp", bufs=3) as fd_pool,
            tc.tile_pool(name="rkp", bufs=3) as rk_pool,
            tc.tile_pool(name="e2p", bufs=2) as e2_pool,
            tc.tile_pool(name="asp", bufs=3) as as_pool,
            tc.tile_pool(name="vp", bufs=10) as v_pool,
            tc.tile_pool(name="hp", bufs=8) as h_pool,
            tc.tile_pool(name="rcp", bufs=4) as rc_pool,
            tc.tile_pool(name="obp", bufs=3) as ob_pool,
            tc.tile_pool(name="pse", bufs=2, space="PSUM") as pse_pool,
            tc.tile_pool(name="psg", bufs=2, space="PSUM") as psg_pool,
        ):
            # ---- constants
            ws0 = cpool.tile([128, 256], F16, tag="ws0")
            ws1 = cpool.tile([128, 256], F16, tag="ws1")
            wd0 = cpool.tile([128, 256], F16, tag="wd0")
            wd1 = cpool.tile([128, 256], F16, tag="wd1")
            nc.sync.dma_start(out=ws0[:], in_=wsrc_d[0])
            nc.sync.dma_start(out=ws1[:], in_=wsrc_d[1])
            nc.sync.dma_start(out=wd0[:], in_=wdst_d[0])
            nc.sync.dma_start(out=wd1[:], in_=wdst_d[1])
            invc_b = cpool.tile([128, 512], F32, tag="invcb")
            nc.gpsimd.dma_start(out=invc_b[:], in_=invc_d[:].to_broadcast((128, 512)))
            invc_v = invc_b[:].rearrange("p (c d) -> p c d", d=256)
            iota_i = cpool.tile([128, 128], I32, tag="iotai")
            nc.gpsimd.iota(iota_i[:], [[1, 128]], channel_multiplier=0)
            iota_f = cpool.tile([128, 128], BF16, tag="iotaf")
            nc.vector.tensor_copy(iota_f[:], iota_i[:])
            if has_bias:
                ones1 = cpool.tile([1, 128], F16, tag="ones1")
                nc.gpsimd.memset(ones1[:], 1.0)
                bs_sb = cpool.tile([1, 256], F16, tag="bs")
                bd_sb = cpool.tile([1, 256], F16, tag="bd")
                nc.sync.dma_start(out=bs_sb[:], in_=bsrc_d[:])
                nc.sync.dma_start(out=bd_sb[:], in_=bdst_d[:])

            import contextlib
            _rep = contextlib.ExitStack()
            if repeat > 1:
                _rep.enter_context(tc.For_i(0, repeat, 1))
            for b in range(nb):
                fs = fs_pool.tile([128, 2, SC], F16, tag="fs")
                fd = fd_pool.tile([128, 2, SC], F16, tag="fd")
                nc.sync.dma_start(
                    out=fs[:], in_=fsT_d[:, :, b * SC:(b + 1) * SC]
                    .rearrange("k p e -> p k e"))
                nc.scalar.dma_start(
                    out=fd[:], in_=fdT_d[:, :, b * SC:(b + 1) * SC]
                    .rearrange("k p e -> p k e"))
                fs0, fs1 = fs[:, 0], fs[:, 1]
                fd0, fd1 = fd[:, 0], fd[:, 1]
                rk = rk_pool.tile([128, CPB], F32, tag="rk")
                nc.sync.dma_start(out=rk[:], in_=rankT_d[:, b * CPB:(b + 1) * CPB])
                e2b = e2_pool.tile([128, GPB * 512], BF16, tag="e2b")
                e2v = e2b[:].rearrange("p (g c d) -> p g c d", c=2, d=256)
                v2s = []
                for mac in range(GPB):     # one macro = one group = 2 chunks
                    pe = pse_pool.tile([128, 1024], F32, tag="pe")  # two banks
                    pev = pe[:].rearrange("p (c b) -> p c b", b=512)
                    v2 = v_pool.tile([128, 514], BF16, tag="v2")
                    v2r = v2[:].rearrange("p (c w) -> p c w", w=257)
                    nc.vector.memset(v2r[:, :, 0:1], 1.0)
                    for m in range(2):
                        j = mac * 2 + m
                        s0, s1 = j * CHUNK, (j + 1) * CHUNK
                        o = pe[:, m * 512:m * 512 + 256]
                        nc.tensor.matmul(out=o, lhsT=fs0[:, s0:s1], rhs=ws0[:],
                                         start=True, stop=False)
                        nc.tensor.matmul(out=o, lhsT=fs1[:, s0:s1], rhs=ws1[:],
                                         start=False, stop=False)
                        if has_bias:
                            nc.tensor.matmul(out=o, lhsT=ones1[:], rhs=bs_sb[:],
                                             start=False, stop=False)
                    # el = psum * (1/c): one strided pass over both chunks
                    nc.vector.tensor_tensor(
                        out=v2r[:, :, 1:257], in0=pev[:, :, 0:256],
                        in1=invc_v, op=ALU.mult)
                    for m in range(2):
                        j = mac * 2 + m
                        s0, s1 = j * CHUNK, (j + 1) * CHUNK
                        o = pe[:, m * 512:m * 512 + 256]
                        nc.tensor.matmul(out=o, lhsT=fd0[:, s0:s1], rhs=wd0[:],
                                         start=False, stop=False)
                        nc.tensor.matmul(out=o, lhsT=fd1[:, s0:s1], rhs=wd1[:],
                                         start=False, stop=not has_bias)
                        if has_bias:
                            nc.tensor.matmul(out=o, lhsT=ones1[:], rhs=bd_sb[:],
                                             start=False, stop=True)
                    # signed score summands: e2 = K*attn_d*lrelu((el+er)_d)
                    if p1 > 0:
                        nc.scalar.activation(
                            e2v[:, mac, :, 0:p1], pev[:, :, 0:p1],
                            AF.Prelu, alpha=NEG_SLOPE)
                    if p1 < 256:
                        nc.scalar.activation(
                            e2v[:, mac, :, p1:256], pev[:, :, p1:256],
                            AF.Prelu, alpha=1.0 / NEG_SLOPE)
                    v2s.append(v2)
                # score*K per chunk = sum of summands; then ex = exp(score)
                asum = as_pool.tile([128, CPB], F32, tag="asum")
                nc.vector.tensor_reduce(
                    out=asum[:], in_=e2b[:].rearrange("p (j d) -> p j d", d=256),
                    axis=AX.X, op=ALU.add)
                exv = as_pool.tile([128, CPB], F32, tag="exv")
                nc.scalar.activation(exv[:], asum[:], AF.Exp, scale=inv_k)
                ob4 = ob_pool.tile([128, GPB * 256], F16, tag="ob4")
                for pair in range(GPB // 2):
                    pg = psg_pool.tile([128, 1024], F32, tag="pg")  # two banks
                    pgv = pg[:].rearrange("p (c b) -> p c b", b=512)
                    for gl in range(2):
                        mac = pair * 2 + gl
                        v2 = v2s[mac]
                        for m in range(2):
                            j = mac * 2 + m
                            hx = h_pool.tile([128, 128], BF16, tag="hx")
                            nc.vector.tensor_scalar(
                                out=hx[:], in0=iota_f[:],
                                scalar1=rk[:, j:j + 1],
                                scalar2=exv[:, j:j + 1],
                                op0=ALU.is_equal, op1=ALU.mult)
                            nc.tensor.matmul(
                                out=pg[:, gl * 512:gl * 512 + 257], lhsT=hx[:],
                                rhs=v2[:, m * 257:(m + 1) * 257],
                                start=(m == 0), stop=(m == 1))
                    rcp = rc_pool.tile([128, 2], F32, tag="rcp")
                    nc.vector.reciprocal_approx_fast(rcp[:], pgv[:, :, 0:1])
                    for gl in range(2):
                        mac = pair * 2 + gl
                        nc.scalar.mul(ob4[:, mac * 256:(mac + 1) * 256],
                                      pg[:, gl * 512 + 1:gl * 512 + 257],
                                      rcp[:, gl:gl + 1])
                g0 = b * GPB
                nc.sync.dma_start(
                    out=dense_d[g0 * 128:(g0 + GPB) * 128, :]
                    .rearrange("(g r) c -> r g c", r=128),
                    in_=ob4[:].rearrange("p (g c) -> p g c", c=256))
            _rep.close()
    nc.compile()
    return nc


# ------------------------------------------------------------------- kernel

def kernel(feat, W_src, b_src, W_dst, b_dst, attn, src, dst, _trace=False):
    global LAST_RESULTS, LAST_NC, LAST_IN_MAPS
    feat = np.asarray(feat, np.float32)
    n_nodes, d_in = feat.shape
    d_out = W_src.shape[1]
    assert d_in == 256 and d_out == 256, "kernel is specialized to D=256"

    p = _prepare(feat, np.asarray(src), np.asarray(dst))
    g_pc, e_slots = p["g_pc"], p["e_slots"]

    has_bias = bool(np.any(b_src) or np.any(b_dst))

    # Signed per-column fold (see module docstring): pos block c=K*attn,
    # neg block c=K*attn*0.2; Prelu alphas 0.2 / 5.0 make
    # Prelu(c*x) == K*attn*leakyrelu(x, 0.2) on both blocks, so
    # score*K = plain sum over all 256 dims.  el = psum*(1/c).
    attn_f = np.asarray(attn, np.float32).reshape(256)
    perm = np.argsort(attn_f < 0, kind="stable")
    p1 = int((attn_f >= 0).sum())
    inv_perm = np.argsort(perm)
    a_perm = attn_f[perm]
    K = float(np.clip(0.02 / max(np.abs(attn_f).min(), 1e-7), 1.0, 1e5))
    a_eff = np.sign(a_perm) * np.maximum(np.abs(a_perm), 1e-7)
    a_eff[a_perm == 0.0] = 1e-7
    c = K * a_eff
    c[p1:] *= NEG_SLOPE
    global LAST_BUILD_ARGS
    LAST_BUILD_ARGS = (g_pc, has_bias, p1, 1.0 / K)
    nc = _build_program(g_pc, has_bias, p1, 1.0 / K)

    feat16 = feat.astype(np.float16)
    wsrc_f = np.asarray(W_src, np.float32)[:, perm] * c[None, :]
    wdst_f = np.asarray(W_dst, np.float32)[:, perm] * c[None, :]
    wsrc16 = np.ascontiguousarray(wsrc_f.astype(np.float16).reshape(2, 128, 256))
    wdst16 = np.ascontiguousarray(wdst_f.astype(np.float16).reshape(2, 128, 256))
    invc_in = np.ascontiguousarray(np.tile(1.0 / c, 2).reshape(1, 512)
                                   .astype(np.float32))

    in_maps = []
    for ci in range(N_CORES):
        sl = slice(ci * e_slots, (ci + 1) * e_slots)
        fs = feat16[p["slot_src"][sl]]          # [e_slots, 256] f16
        fd = feat16[p["slot_dst"][sl]]
        fsT = np.ascontiguousarray(fs.T).reshape(2, 128, e_slots)
        fdT = np.ascontiguousarray(fd.T).reshape(2, 128, e_slots)
        rankT = np.ascontiguousarray(
            p["slot_rank"][sl].reshape(g_pc * 2, 128).T)
        m = {"fsT": fsT, "fdT": fdT, "rankT": rankT,
             "wsrc": wsrc16, "wdst": wdst16, "invc": invc_in}
        if has_bias:
            m["bsrc"] = np.ascontiguousarray(
                (np.asarray(b_src, np.float32)[perm] * c)
                .astype(np.float16).reshape(1, 256))
            m["bdst"] = np.ascontiguousarray(
                (np.asarray(b_dst, np.float32)[perm] * c)
                .astype(np.float16).reshape(1, 256))
        in_maps.append(m)

    if not _trace:
        # The axon NTFF-trace hook is unavailable in this container; make sure
        # an externally-set BASS_TRACE can't route us into that path.
        import os
        os.environ["BASS_NEVER_TRACE"] = "1"
    res = run_bass_kernel_spmd(nc, in_maps, core_ids=list(range(N_CORES)),
                               trace=_trace)
    LAST_RESULTS, LAST_NC, LAST_IN_MAPS = res, nc, in_maps

    out = np.zeros((n_nodes, 256), np.float32)
    run_core, run_pos, run_node = p["run_core"], p["run_pos"], p["run_node"]
    for ci in range(N_CORES):
        dense = res.results[ci]["dense"]
        mask = run_core == ci
        if not mask.any():
            continue
        rows = dense[run_pos[mask]][:, inv_perm]   # undo the attn column sort
        if p["need_accum"]:
            np.add.at(out, run_node[mask], rows)
        else:
            out[run_node[mask]] = rows
    return out


# revision 35
# speedup vs baseline: 8.3669x; 8.3669x over previous
"""GATv2Conv forward on 8 Trainium2 NeuronCores (Bass/Tile).

Strategy
--------
Edges are sorted by destination node and packed into "groups" of at most
S=256 edge slots / at most 128 distinct destinations, such that every
destination's edge run lies entirely inside one group.  Groups are split
evenly across the 8 cores, so all softmax segments and output rows are
core-local: no collectives are needed.

Host preprocessing is index plumbing + dtype conversion only: it gathers
feat[src]/feat[dst] per edge slot (fp16, K-major transposed) and ships
the raw fp16 weights.

Per core (SPMD, identical program; per-core data differs):
  - per group, a 2-bank PSUM pair [128, 2, 256] (chunk-major, stride 512):
    4 fp16 el matmuls, ONE strided ACT copy evacuates el into the bf16
    aggregation operand V=[1|el], then 4 er matmuls accumulate in place,
  - score: a custom DVE op (registered at import, per-NEFF uop table)
      body = max(x, 0.2*x) * attn_d,  accum = sum over d
    reads the PSUM pair once per chunk and emits score[e] directly —
    no Prelu pass, no weight folding, no separate reduce,
  - ONE batched ACT Exp per 4-group batch,
  - per group: bf16 one-hot scatter-add matmul
    psum[rank,:] += Hx^T @ V with Hx[e,k] = (rank_e==k)*ex_e
    (Hx built by one DVE tensor_scalar: is_equal(iota,rank)*ex);
    segment sums land in column 0.  pg tiles are paired (2 groups / 2
    banks) so the reciprocal is one strided DVE op per pair,
  - out rows = psum[:,1:257] * rcp  (ACT copy-mul, fp16 out).

The batch loop is software-pipelined one batch deep (batch b's
GEMM/evac/score head is emitted before batch b-1's exp/scatter tail) so
the in-order DVE/ACT queues never stall on the exp round-trip.

Host unshards by scattering dense group rows to their global node ids
(pure indexing; all arithmetic on-device).
"""

import math
import numpy as np

import concourse.bass as bass
import concourse.mybir as mybir
import concourse.tile as tile
from concourse import bacc
from concourse.bass_utils import run_bass_kernel_spmd

F32 = mybir.dt.float32
F16 = mybir.dt.float16
BF16 = mybir.dt.bfloat16
F8E4 = mybir.dt.float8e4
I32 = mybir.dt.int32
FEAT_SCALE = 16.0    # fp8 path: feat quantized at x16
W_SCALE = 64.0       # fp8 path: weights quantized at x64
PSUM_SCALE = FEAT_SCALE * W_SCALE
AF = mybir.ActivationFunctionType
ALU = mybir.AluOpType
AX = mybir.AxisListType

N_CORES = 8
S = 256            # edge slots per group (2 chunks of 128)
CHUNK = 128
GPB = 4            # groups per batch
CPB = 2 * GPB      # chunks per batch
SC = S * GPB       # edge slots per batch
NEG_SLOPE = 0.2

LAST_RESULTS = None  # BassKernelResults of the most recent run (for test.py)
LAST_NC = None       # compiled Bacc program of the most recent run
LAST_IN_MAPS = None  # per-core input dicts of the most recent run
LAST_BUILD_ARGS = None  # (g_pc, has_bias) for rebuilds


# ----------------------------------------------- custom DVE op registration

def _register_lrelu_dot():
    """out = max(x, 0.2x)*attn elementwise; accum_out = sum over free dim.

    One PSUM pass computes the whole GATv2 edge score.  The uop program is
    written into the per-NEFF DVE table at compile time; shas are computed
    here exactly as DveOp.compile does, so they can never drift.
    """
    from concourse.dve_ops import OPS, DveOp, get_dve_sub_opcode
    import concourse.dve_ops as _dops
    from concourse.dve_spec import Spec, Src0, Src1, C1, maxx, lower
    from concourse.dve_uop import DveOpSpec

    for op in OPS:
        if op.name == "LRELU_DOT_ANT":
            return op

    def _ref(in0, in1, s0, s1, imm2):
        x = np.nan_to_num(in0.astype(np.float32), nan=0.0,
                          posinf=np.inf, neginf=-np.inf)
        b = (np.maximum(x, x * s1) * in1).astype(np.float32)
        return b, b.reshape(b.shape[0], -1).sum(axis=-1, keepdims=True)

    spec = Spec(body=maxx(Src0, Src0 * C1) * Src1,
                accum=_dops.add, reference=_ref)
    shas = {}
    op = DveOp("LRELU_DOT_ANT", spec, subdim=False, uops_sha=shas)
    OPS.append(op)   # position assigns the table row; append before compile
    # module-level lookups are built at import; extend them for the new row
    _dops._SUB_OPCODE_FOR_NAME[op.name] = (
        _dops._CUSTOM_DVE_ROW_BASE + len(OPS) - 1)
    assert _dops._SUB_OPCODE_FOR_NAME[op.name] < 0x20
    _dops.CUSTOM_DVE_SPECS[op.name] = spec
    has_src1 = getattr(_dops, "has_src1", None)
    if has_src1 is None:
        from concourse.dve_spec import has_src1
    for ver in ("v3", "v4"):
        compiled = DveOpSpec(
            name=op.name, opcode=get_dve_sub_opcode(op.name),
            uops=lower(spec, ver=ver), rd1_en=has_src1(spec))
        shas[ver] = compiled.sha(ver)
    return op


LRELU_DOT = _register_lrelu_dot()


# ----------------------------------------------------------------- host prep

def _pack_runs(counts, max_slots=S, max_nodes=128):
    """Greedily pack whole runs (same-dst edge blocks) into groups."""
    n = len(counts)
    grp = np.empty(n, np.int64)
    rank = np.empty(n, np.int64)
    g = used = nodes = 0
    for i in range(n):
        c = counts[i]
        if used + c > max_slots or nodes >= max_nodes:
            g += 1
            used = 0
            nodes = 0
        grp[i] = g
        rank[i] = nodes
        used += c
        nodes += 1
    return grp, rank, g + 1


def _prepare(feat, src, dst):
    """Sort edges by dst, pack into groups, build per-core input arrays."""
    E = dst.shape[0]
    order = np.argsort(dst, kind="stable")
    sd = dst[order].astype(np.int64)
    ss = src[order].astype(np.int64)

    uniq, counts = np.unique(sd, return_counts=True)
    # split pathological runs longer than S so packing can't fail
    need_accum = bool((counts > S).any())
    if need_accum:
        new_uniq, new_counts = [], []
        for u, c in zip(uniq, counts):
            while c > S:
                new_uniq.append(u)
                new_counts.append(S)
                c -= S
            new_uniq.append(u)
            new_counts.append(c)
        uniq = np.array(new_uniq, np.int64)
        counts = np.array(new_counts, np.int64)

    grp, rank, g_tot = _pack_runs(counts)
    n_runs = len(counts)

    starts = np.zeros(n_runs, np.int64)
    np.cumsum(counts[:-1], out=starts[1:])
    grp_first_run = np.searchsorted(grp, np.arange(g_tot))
    grp_start_edge = starts[grp_first_run]

    run_of_edge = np.repeat(np.arange(n_runs), counts)
    e_grp = grp[run_of_edge]
    e_rank = rank[run_of_edge]
    e_slot = e_grp * S + np.arange(E) - grp_start_edge[e_grp]

    g_pc = math.ceil(g_tot / N_CORES)
    g_pc = math.ceil(g_pc / GPB) * GPB          # multiple of GPB per core
    e_slots = g_pc * S
    total_slots = N_CORES * e_slots

    slot_src = np.zeros(total_slots, np.int64)
    slot_dst = np.zeros(total_slots, np.int64)
    slot_rank = np.full(total_slots, -1.0, np.float32)
    slot_src[e_slot] = ss
    slot_dst[e_slot] = sd
    slot_rank[e_slot] = e_rank

    # unshard info per run
    run_core = grp // g_pc
    run_pos = (grp % g_pc) * 128 + rank        # row in the core's dense output

    return dict(
        g_pc=g_pc, e_slots=e_slots, need_accum=need_accum,
        slot_src=slot_src, slot_dst=slot_dst, slot_rank=slot_rank,
        run_core=run_core, run_pos=run_pos, run_node=uniq,
    )


# ------------------------------------------------------------ device program

def _build_program(n_g, has_bias, repeat=1, pse_bufs=4, psg_bufs=1,
                   v_bufs=10, h_bufs=8, fs_bufs=3, pg_single=False,
                   interleave=False, stagger=1, ones_on_pool=True,
                   unified_psum=True, fp8=False):
    assert not (fp8 and has_bias), "fp8 path does not implement bias"
    e_slots = n_g * S
    nb = n_g // GPB
    nc = bacc.Bacc("TRN2", target_bir_lowering=False, debug=False,
                   num_devices=N_CORES)
    if fp8:
        fsT_d = nc.dram_tensor("fsT", [128, 2, e_slots], F8E4,
                               kind="ExternalInput").ap()
        fdT_d = nc.dram_tensor("fdT", [128, 2, e_slots], F8E4,
                               kind="ExternalInput").ap()
    else:
        fsT_d = nc.dram_tensor("fsT", [2, 128, e_slots], F16,
                               kind="ExternalInput").ap()
        fdT_d = nc.dram_tensor("fdT", [2, 128, e_slots], F16,
                               kind="ExternalInput").ap()
    rankT_d = nc.dram_tensor("rankT", [128, n_g * 2], F32, kind="ExternalInput").ap()
    wdt = F8E4 if fp8 else F16
    wshape = [128, 2, 256] if fp8 else [2, 128, 256]
    wsrc_d = nc.dram_tensor("wsrc", wshape, wdt, kind="ExternalInput").ap()
    wdst_d = nc.dram_tensor("wdst", wshape, wdt, kind="ExternalInput").ap()
    attn_d = nc.dram_tensor("attnv", [1, 256], F32, kind="ExternalInput").ap()
    if has_bias:
        bsrc_d = nc.dram_tensor("bsrc", [1, 256], F16, kind="ExternalInput").ap()
        bdst_d = nc.dram_tensor("bdst", [1, 256], F16, kind="ExternalInput").ap()
    dense_d = nc.dram_tensor("dense", [n_g * 128, 256], F16,
                             kind="ExternalOutput").ap()

    with tile.TileContext(nc) as tc:
        with (
            tc.tile_pool(name="const", bufs=1) as cpool,
            tc.tile_pool(name="fsp", bufs=fs_bufs) as fs_pool,
            tc.tile_pool(name="fdp", bufs=fs_bufs) as fd_pool,
            tc.tile_pool(name="rkp", bufs=3) as rk_pool,
            tc.tile_pool(name="asp", bufs=5) as as_pool,
            tc.tile_pool(name="scr", bufs=3) as scr_pool,
            tc.tile_pool(name="vp", bufs=v_bufs) as v_pool,
            tc.tile_pool(name="hp", bufs=h_bufs) as h_pool,
            tc.tile_pool(name="rcp", bufs=4) as rc_pool,
            tc.tile_pool(name="obp", bufs=3) as ob_pool,
            tc.tile_pool(name="pse", bufs=pse_bufs, space="PSUM") as pse_pool,
            tc.tile_pool(name="psg", bufs=psg_bufs, space="PSUM") as psg_pool,
        ):
            # ---- constants
            if fp8:
                ws8 = cpool.tile([128, 2, 256], F8E4, tag="ws8")
                wd8 = cpool.tile([128, 2, 256], F8E4, tag="wd8")
                nc.sync.dma_start(out=ws8[:], in_=wsrc_d[:])
                nc.sync.dma_start(out=wd8[:], in_=wdst_d[:])
                ws0 = ws1 = wd0 = wd1 = None
            else:
                ws0 = cpool.tile([128, 256], F16, tag="ws0")
                ws1 = cpool.tile([128, 256], F16, tag="ws1")
                wd0 = cpool.tile([128, 256], F16, tag="wd0")
                wd1 = cpool.tile([128, 256], F16, tag="wd1")
                nc.sync.dma_start(out=ws0[:], in_=wsrc_d[0])
                nc.sync.dma_start(out=ws1[:], in_=wsrc_d[1])
                nc.sync.dma_start(out=wd0[:], in_=wdst_d[0])
                nc.sync.dma_start(out=wd1[:], in_=wdst_d[1])
            attn_b = cpool.tile([128, 256], F32, tag="attnb")
            nc.gpsimd.dma_start(out=attn_b[:], in_=attn_d[:].to_broadcast((128, 256)))
            iota_i = cpool.tile([128, 128], I32, tag="iotai")
            nc.gpsimd.iota(iota_i[:], [[1, 128]], channel_multiplier=0)
            iota_f = cpool.tile([128, 128], BF16, tag="iotaf")
            nc.vector.tensor_copy(iota_f[:], iota_i[:])
            if has_bias:
                ones1 = cpool.tile([1, 128], F16, tag="ones1")
                nc.gpsimd.memset(ones1[:], 1.0)
                bs_sb = cpool.tile([1, 256], F16, tag="bs")
                bd_sb = cpool.tile([1, 256], F16, tag="bd")
                nc.sync.dma_start(out=bs_sb[:], in_=bsrc_d[:])
                nc.sync.dma_start(out=bd_sb[:], in_=bdst_d[:])

            def start_batch(b):
                fdt = F8E4 if fp8 else F16
                fs = fs_pool.tile([128, 2, SC], fdt, tag="fs")
                fd = fd_pool.tile([128, 2, SC], fdt, tag="fd")
                if fp8:
                    nc.sync.dma_start(
                        out=fs[:], in_=fsT_d[:, :, b * SC:(b + 1) * SC])
                    nc.scalar.dma_start(
                        out=fd[:], in_=fdT_d[:, :, b * SC:(b + 1) * SC])
                else:
                    nc.sync.dma_start(
                        out=fs[:], in_=fsT_d[:, :, b * SC:(b + 1) * SC]
                        .rearrange("k p e -> p k e"))
                    nc.scalar.dma_start(
                        out=fd[:], in_=fdT_d[:, :, b * SC:(b + 1) * SC]
                        .rearrange("k p e -> p k e"))
                rk = rk_pool.tile([128, CPB], F32, tag="rk")
                nc.sync.dma_start(out=rk[:], in_=rankT_d[:, b * CPB:(b + 1) * CPB])
                sc = as_pool.tile([128, CPB], F32, tag="sc")
                return dict(b=b, fs=fs, fd=fd, rk=rk, sc=sc, v2s=[])

            DR = mybir.MatmulPerfMode.DoubleRow

            def emit_el(bs, mac):
                """el matmuls + evacuation for group `mac` of batch `bs`."""
                pe = pse_pool.tile([128, 1024], F32, tag="pe")  # two banks
                pev = pe[:].rearrange("p (c b) -> p c b", b=512)
                v2 = v_pool.tile([128, 514], BF16, tag="v2")
                v2r = v2[:].rearrange("p (c w) -> p c w", w=257)
                (nc.gpsimd if ones_on_pool else nc.vector).memset(
                    v2r[:, :, 0:1], 1.0)
                if fp8:
                    fs = bs["fs"]
                    for m in range(2):
                        j = mac * 2 + m
                        s0, s1 = j * CHUNK, (j + 1) * CHUNK
                        o = pe[:, m * 512:m * 512 + 256]
                        nc.tensor.matmul(out=o, lhsT=fs[:, :, s0:s1],
                                         rhs=ws8[:], perf_mode=DR,
                                         start=True, stop=False)
                else:
                    fs0, fs1 = bs["fs"][:, 0], bs["fs"][:, 1]
                    for m in range(2):
                        j = mac * 2 + m
                        s0, s1 = j * CHUNK, (j + 1) * CHUNK
                        o = pe[:, m * 512:m * 512 + 256]
                        nc.tensor.matmul(out=o, lhsT=fs0[:, s0:s1], rhs=ws0[:],
                                         start=True, stop=False)
                        nc.tensor.matmul(out=o, lhsT=fs1[:, s0:s1], rhs=ws1[:],
                                         start=False, stop=False)
                        if has_bias:
                            nc.tensor.matmul(out=o, lhsT=ones1[:], rhs=bs_sb[:],
                                             start=False, stop=False)
                # el evacuation: one strided ACT copy over both chunks
                nc.scalar.activation(v2r[:, :, 1:257], pev[:, :, 0:256],
                                     AF.Copy, scale=1.0 / PSUM_SCALE if fp8
                                     else 1.0)
                bs["v2s"].append(v2)
                return (bs, mac, pe, pev)

            def emit_er_score(g):
                """er matmuls + score for a group emitted by emit_el."""
                bs, mac, pe, pev = g
                if fp8:
                    fd = bs["fd"]
                    for m in range(2):
                        j = mac * 2 + m
                        s0, s1 = j * CHUNK, (j + 1) * CHUNK
                        o = pe[:, m * 512:m * 512 + 256]
                        nc.tensor.matmul(out=o, lhsT=fd[:, :, s0:s1],
                                         rhs=wd8[:], perf_mode=DR,
                                         start=False, stop=True)
                else:
                    fd0, fd1 = bs["fd"][:, 0], bs["fd"][:, 1]
                    for m in range(2):
                        j = mac * 2 + m
                        s0, s1 = j * CHUNK, (j + 1) * CHUNK
                        o = pe[:, m * 512:m * 512 + 256]
                        nc.tensor.matmul(out=o, lhsT=fd0[:, s0:s1], rhs=wd0[:],
                                         start=False, stop=False)
                        nc.tensor.matmul(out=o, lhsT=fd1[:, s0:s1], rhs=wd1[:],
                                         start=False, stop=not has_bias)
                        if has_bias:
                            nc.tensor.matmul(out=o, lhsT=ones1[:], rhs=bd_sb[:],
                                             start=False, stop=True)
                # score[e] = sum_d max(x, 0.2x)*attn_d — one PSUM pass
                for m in range(2):
                    j = mac * 2 + m
                    junk = scr_pool.tile([128, 256], BF16, tag="junk")
                    nc.vector._custom_dve(
                        LRELU_DOT, out=junk[:], in0=pev[:, m, 0:256],
                        in1=attn_b[:], s1=NEG_SLOPE,
                        accum_out=bs["sc"][:, j:j + 1])

            def emit_head(b, tail_cb=None):
                bs = start_batch(b)
                for mac in range(GPB):     # one macro = one group = 2 chunks
                    if tail_cb is not None:
                        tail_cb(mac)
                    g = emit_el(bs, mac)
                    emit_er_score(g)
                return (b, bs["v2s"], bs["sc"], bs["rk"], {})

            def emit_tail(state, stage=None):
                """stage None: everything.  0: exp.  1: pair 0.  2: pair 1+DMA.
                Staged emission interleaves the previous batch's tail between
                the current batch's head groups (PE/ACT/DVE streams stay fed)."""
                b, v2s, sc, rk = state[:4]
                cache = state[4]
                if stage in (None, 0):
                    exv = as_pool.tile([128, CPB], F32, tag="exv")
                    nc.scalar.activation(exv[:], sc[:], AF.Exp,
                                         scale=1.0 / PSUM_SCALE if fp8 else 1.0)
                    ob4 = ob_pool.tile([128, GPB * 256], F16, tag="ob4")
                    cache["exv"], cache["ob4"] = exv, ob4
                    if stage == 0:
                        return
                exv, ob4 = cache["exv"], cache["ob4"]
                if pg_single:
                    for mac in range(GPB):
                        pg = psg_pool.tile([128, 512], F32, tag="pg")
                        v2 = v2s[mac]
                        for m in range(2):
                            j = mac * 2 + m
                            hx = h_pool.tile([128, 128], BF16, tag="hx")
                            nc.vector.tensor_scalar(
                                out=hx[:], in0=iota_f[:],
                                scalar1=rk[:, j:j + 1],
                                scalar2=exv[:, j:j + 1],
                                op0=ALU.is_equal, op1=ALU.mult)
                            nc.tensor.matmul(
                                out=pg[:, 0:257], lhsT=hx[:],
                                rhs=v2[:, m * 257:(m + 1) * 257],
                                start=(m == 0), stop=(m == 1))
                        rcp = rc_pool.tile([128, 1], F32, tag="rcp")
                        nc.vector.reciprocal_approx_fast(rcp[:], pg[:, 0:1])
                        nc.scalar.mul(ob4[:, mac * 256:(mac + 1) * 256],
                                      pg[:, 1:257], rcp[:, 0:1])
                    g0 = b * GPB
                    nc.sync.dma_start(
                        out=dense_d[g0 * 128:(g0 + GPB) * 128, :]
                        .rearrange("(g r) c -> r g c", r=128),
                        in_=ob4[:].rearrange("p (g c) -> p g c", c=256))
                    return
                pairs = ((0,) if stage == 1 else (1,) if stage == 2
                         else range(GPB // 2))
                for pair in pairs:
                    if unified_psum:   # reuse just-released pe banks
                        pg = pse_pool.tile([128, 1024], F32, tag="pe")
                    else:
                        pg = psg_pool.tile([128, 1024], F32, tag="pg")
                    pgv = pg[:].rearrange("p (c b) -> p c b", b=512)
                    for gl in range(2):
                        mac = pair * 2 + gl
                        v2 = v2s[mac]
                        for m in range(2):
                            j = mac * 2 + m
                            hx = h_pool.tile([128, 128], BF16, tag="hx")
                            nc.vector.tensor_scalar(
                                out=hx[:], in0=iota_f[:],
                                scalar1=rk[:, j:j + 1],
                                scalar2=exv[:, j:j + 1],
                                op0=ALU.is_equal, op1=ALU.mult)
                            nc.tensor.matmul(
                                out=pg[:, gl * 512:gl * 512 + 257], lhsT=hx[:],
                                rhs=v2[:, m * 257:(m + 1) * 257],
                                start=(m == 0), stop=(m == 1))
                    rcp = rc_pool.tile([128, 2], F32, tag="rcp")
                    nc.vector.reciprocal_approx_fast(rcp[:], pgv[:, :, 0:1])
                    for gl in range(2):
                        mac = pair * 2 + gl
                        nc.scalar.mul(ob4[:, mac * 256:(mac + 1) * 256],
                                      pg[:, gl * 512 + 1:gl * 512 + 257],
                                      rcp[:, gl:gl + 1])
                if stage == 1:
                    return
                g0 = b * GPB
                nc.sync.dma_start(
                    out=dense_d[g0 * 128:(g0 + GPB) * 128, :]
                    .rearrange("(g r) c -> r g c", r=128),
                    in_=ob4[:].rearrange("p (g c) -> p g c", c=256))

            import contextlib
            _rep = contextlib.ExitStack()
            if repeat > 1:
                _rep.enter_context(tc.For_i(0, repeat, 1))
            state = None
            if stagger:
                from collections import deque
                pend = deque()

                def flush_pend(keep):
                    while len(pend) > keep:
                        emit_er_score(pend.popleft())

                for b in range(nb):
                    bs = start_batch(b)
                    for mac in range(GPB):
                        if mac == 0 and state is not None:
                            flush_pend(0)      # finish batch b-1's scores
                            emit_tail(state, stage=0)
                        elif mac == 2 and state is not None:
                            emit_tail(state, stage=1)
                        pend.append(emit_el(bs, mac))
                        flush_pend(int(stagger))
                    if state is not None:
                        emit_tail(state, stage=2)
                    state = (b, bs["v2s"], bs["sc"], bs["rk"], {})
                flush_pend(0)
                emit_tail(state)
            elif interleave:
                def make_cb(st):
                    if st is None:
                        return None

                    def cb(mac):
                        if mac == 0:
                            emit_tail(st, stage=0)
                        elif mac == 2:
                            emit_tail(st, stage=1)
                    return cb

                for b in range(nb):
                    new_state = emit_head(b, tail_cb=make_cb(state))
                    if state is not None:
                        emit_tail(state, stage=2)
                    state = new_state
                emit_tail(state)
            else:
                for b in range(nb):
                    new_state = emit_head(b)
                    if state is not None:
                        emit_tail(state)
                    state = new_state
                emit_tail(state)
            _rep.close()
    nc.compile()
    return nc


# ------------------------------------------------------------------- kernel

def kernel(feat, W_src, b_src, W_dst, b_dst, attn, src, dst, _trace=False,
           _fp8=True):
    global LAST_RESULTS, LAST_NC, LAST_IN_MAPS
    import ml_dtypes
    feat = np.asarray(feat, np.float32)
    n_nodes, d_in = feat.shape
    d_out = W_src.shape[1]
    assert d_in == 256 and d_out == 256, "kernel is specialized to D=256"

    p = _prepare(feat, np.asarray(src), np.asarray(dst))
    g_pc, e_slots = p["g_pc"], p["e_slots"]

    has_bias = bool(np.any(b_src) or np.any(b_dst))
    fp8 = _fp8 and not has_bias
    f8 = ml_dtypes.float8_e4m3

    global LAST_BUILD_ARGS
    LAST_BUILD_ARGS = (g_pc, has_bias)
    LAST_BUILD_KW = {"fp8": fp8}
    globals()["LAST_BUILD_KW"] = LAST_BUILD_KW
    nc = _build_program(g_pc, has_bias, fp8=fp8)

    if fp8:
        feat_q = np.clip(feat * FEAT_SCALE, -240, 240).astype(f8)
        wsrc_q = np.clip(np.asarray(W_src, np.float32) * W_SCALE,
                         -240, 240).astype(f8)
        wdst_q = np.clip(np.asarray(W_dst, np.float32) * W_SCALE,
                         -240, 240).astype(f8)
        # [256k, 256n] -> [128, 2, 256]: row k+128j lands at [k, j, :]
        wsrc_in = np.ascontiguousarray(
            wsrc_q.reshape(2, 128, 256).transpose(1, 0, 2))
        wdst_in = np.ascontiguousarray(
            wdst_q.reshape(2, 128, 256).transpose(1, 0, 2))
    else:
        feat_q = feat.astype(np.float16)
        wsrc_in = np.ascontiguousarray(
            np.asarray(W_src, np.float32).astype(np.float16)
            .reshape(2, 128, 256))
        wdst_in = np.ascontiguousarray(
            np.asarray(W_dst, np.float32).astype(np.float16)
            .reshape(2, 128, 256))
    attn_in = np.ascontiguousarray(
        np.asarray(attn, np.float32).reshape(1, 256))

    in_maps = []
    for ci in range(N_CORES):
        sl = slice(ci * e_slots, (ci + 1) * e_slots)
        fs = feat_q[p["slot_src"][sl]]          # [e_slots, 256]
        fd = feat_q[p["slot_dst"][sl]]
        if fp8:
            # [e, 256k] -> [128, 2, e]: logical k+128j at [k, j, e]
            fsT = np.ascontiguousarray(
                fs.T.reshape(2, 128, e_slots).transpose(1, 0, 2))
            fdT = np.ascontiguousarray(
                fd.T.reshape(2, 128, e_slots).transpose(1, 0, 2))
        else:
            fsT = np.ascontiguousarray(fs.T).reshape(2, 128, e_slots)
            fdT = np.ascontiguousarray(fd.T).reshape(2, 128, e_slots)
        rankT = np.ascontiguousarray(
            p["slot_rank"][sl].reshape(g_pc * 2, 128).T)
        m = {"fsT": fsT, "fdT": fdT, "rankT": rankT,
             "wsrc": wsrc_in, "wdst": wdst_in, "attnv": attn_in}
        if has_bias:
            m["bsrc"] = np.ascontiguousarray(
                np.asarray(b_src, np.float32).astype(np.float16).reshape(1, 256))
            m["bdst"] = np.ascontiguousarray(
                np.asarray(b_dst, np.float32).astype(np.float16).reshape(1, 256))
        in_maps.append(m)

    if not _trace:
        # The axon NTFF-trace hook is unavailable in this container; make sure
        # an externally-set BASS_TRACE can't route us into that path.
        import os
        os.environ["BASS_NEVER_TRACE"] = "1"
    res = run_bass_kernel_spmd(nc, in_maps, core_ids=list(range(N_CORES)),
                               trace=_trace)
    LAST_RESULTS, LAST_NC, LAST_IN_MAPS = res, nc, in_maps

    out = np.zeros((n_nodes, 256), np.float32)
    run_core, run_pos, run_node = p["run_core"], p["run_pos"], p["run_node"]
    for ci in range(N_CORES):
        dense = res.results[ci]["dense"]
        mask = run_core == ci
        if not mask.any():
            continue
        rows = dense[run_pos[mask]]
        if p["need_accum"]:
            np.add.at(out, run_node[mask], rows)
        else:
            out[run_node[mask]] = rows
    return out
